# revision 12
# baseline (speedup 1.0000x reference)
"""Fastformer (additive attention) Bass kernel for Trainium2, 8-core data-parallel.

Math (per batch element b, derived from the reference by algebraic collapse):
    A_q   = Wq @ Wqa                                   [768, 12]   (host)
    s_q   = x @ A_q + log_mask                         [S, 12]
    e_q   = exp(s_q / 8);  den_q = sum_s e_q           [12]
    xq    = (e_q^T @ x) / (den_q + 1e-8)               [12, 768]
    q_ctx = diag-blocks of (xq @ Wq)                   [768]  (flat)
    A_k   = Wk @ (q_ctx * Wka)                         [768, 12]
    ... same pooling again -> kc0, k_ctx = q_ctx * kc0 [768]
    M     = Wq + concat_h(Wq[:,h] @ (k_ctx[h] . Wo))   [768, 768]
    out   = x @ M                                      [S, 768]

All big matmuls run in fp16 (fp32 accumulation in PSUM); the small weight-side
ops stay fp32. Sharding: batch b -> core b (B == n_cores == 8).
"""
import math
from contextlib import ExitStack

import numpy as np

import concourse.bass as bass
import concourse.bacc as bacc
import concourse.tile as tile
import concourse.mybir as mybir

F16 = mybir.dt.float16
F32 = mybir.dt.float32

B, S, F, H, D = 8, 4096, 768, 12, 64
P = 128
NF = F // P          # 6 feature chunks
NS = S // P          # 32 seq chunks of 128
NC = S // 512        # 8 seq chunks of 512


def _set_seqlen(s):
    global S, NS, NC
    S, NS, NC = s, s // P, s // 512
N_CORES = 8
EXP_SCALE = 1.0 / math.sqrt(D)   # 1/8

_prog_cache = {}


def _emit_pool_pass(nc, tc, pools, cst, A_chunks, tag_prefix):
    """Scores -> exp -> transpose -> weighted-sum pass.

    A_chunks: function j -> stationary AP [128, 12] (fp16) for feature chunk j.
    Returns (xw_psum [12,768] f32 AP, inv_den [12,1] f32 AP).
    """
    psA, psW, ework = pools["psA"], pools["psW"], pools["ework"]
    xT_sb, lm_sb, ones_sb, id_sb, x_sb = (
        cst["xT_sb"], cst["lm_sb"], cst["ones_sb"], cst["id_sb"], cst["x_sb"])

    den_parts = ework.tile([12, NC], F32, tag=f"{tag_prefix}denp")
    eT = []
    for c in range(NC):
        sc = psA.tile([12, 512], F32, tag="sc")
        for j in range(NF):
            nc.tensor.matmul(sc[:], A_chunks(j), xT_sb[j][:, 512 * c:512 * (c + 1)],
                             start=(j == 0), stop=False)
        nc.tensor.matmul(sc[:], ones_sb[:], lm_sb[:, 512 * c:512 * (c + 1)],
                         start=False, stop=True)
        et = ework.tile([12, 512], F16, tag=f"eT{c}")
        nc.scalar.activation(et[:], sc[:], mybir.ActivationFunctionType.Exp,
                             scale=EXP_SCALE, accum_out=den_parts[:, c:c + 1])
        eT.append(et)

    den = ework.tile([12, 1], F32, tag=f"{tag_prefix}den")
    nc.vector.tensor_reduce(den[:], den_parts[:], axis=mybir.AxisListType.X,
                            op=mybir.AluOpType.add)
    inv = ework.tile([12, 1], F32, tag=f"{tag_prefix}inv")
    nc.vector.tensor_scalar_add(inv[:], den[:], 1e-8)
    nc.vector.reciprocal(inv[:], inv[:])

    # transpose e to [s, 12] chunks
    e_sb = []
    for i in range(NS):
        tp = pools["psB"].tile([P, 12], F16, tag="tp")
        src = eT[i // 4][:, P * (i % 4):P * (i % 4 + 1)]
        nc.tensor.transpose(tp[:], src, id_sb[:])
        e = ework.tile([P, 12], F16, tag=f"e{i}")
        nc.vector.tensor_copy(e[:], tp[:])
        e_sb.append(e)

    # xw = e^T @ x accumulated over all 32 chunks
    xw = psW.tile([12, F], F32, tag="wide")
    for i in range(NS):
        nc.tensor.matmul(xw[:, 0:512], e_sb[i][:], x_sb[i][:, 0:512],
                         start=(i == 0), stop=(i == NS - 1))
        nc.tensor.matmul(xw[:, 512:F], e_sb[i][:], x_sb[i][:, 512:F],
                         start=(i == 0), stop=(i == NS - 1))
    return xw, inv


def _emit_ctx_extract(nc, tc, pools, cst, xw, inv, W_sb, tag_prefix):
    """xw,inv -> normalized xq (f16, transposed chunks) -> G = xq @ W -> ctx col.

    Returns ctx [128, 6] f32 tile (flat [768] ctx vector, col j = f-chunk j).
    """
    ework, psW, psB = pools["ework"], pools["psW"], pools["psB"]
    id_sb = cst["id_sb"]

    xq = ework.tile([12, F], F16, tag=f"{tag_prefix}xq")
    nc.vector.tensor_scalar_mul(xq[:], xw[:], inv[:])

    xqT = ework.tile([P, 12 * NF], F16, tag=f"{tag_prefix}xqT")
    for j in range(NF):
        tp = psB.tile([P, 12], F16, tag="tp")
        nc.tensor.transpose(tp[:], xq[:, P * j:P * (j + 1)], id_sb[:])
        nc.vector.tensor_copy(xqT[:, 12 * j:12 * (j + 1)], tp[:])

    G = psW.tile([12, F], F32, tag="wide")
    for j in range(NF):
        nc.tensor.matmul(G[:, 0:512], xqT[:, 12 * j:12 * (j + 1)],
                         W_sb[j][:, 0:512], start=(j == 0), stop=(j == NF - 1))
        nc.tensor.matmul(G[:, 512:F], xqT[:, 12 * j:12 * (j + 1)],
                         W_sb[j][:, 512:F], start=(j == 0), stop=(j == NF - 1))

    G16 = ework.tile([12, F], F16, tag=f"{tag_prefix}G16")
    nc.vector.tensor_copy(G16[:], G[:])

    ctx = ework.tile([P, NF], F32, tag=f"{tag_prefix}ctx")
    for m in range(NF):
        tp = psB.tile([P, 12], F16, tag="tp")
        nc.tensor.transpose(tp[:], G16[:, P * m:P * (m + 1)], id_sb[:])
        nc.vector.tensor_copy(ctx[0:64, m:m + 1], tp[0:64, 2 * m:2 * m + 1])
        nc.vector.tensor_copy(ctx[64:P, m:m + 1], tp[64:P, 2 * m + 1:2 * m + 2])
    return ctx


def build_program(stage=4):
    nc = bacc.Bacc(trn_type="TRN2", target_bir_lowering=False)

    xT_d = nc.dram_tensor("xT", [F, S], F16, kind="ExternalInput")
    x_d = nc.dram_tensor("x", [S, F], F16, kind="ExternalInput")
    lm_d = nc.dram_tensor("lm", [1, S], F16, kind="ExternalInput")
    Aq_d = nc.dram_tensor("Aq", [F, 12], F16, kind="ExternalInput")
    Wq_d = nc.dram_tensor("Wq", [F, F], F16, kind="ExternalInput")
    Wqt_d = nc.dram_tensor("Wqt", [F, F], F16, kind="ExternalInput")
    Wk_d = nc.dram_tensor("Wk", [F, F], F16, kind="ExternalInput")
    Wkt_d = nc.dram_tensor("Wkt", [F, F], F16, kind="ExternalInput")
    Wka_d = nc.dram_tensor("Wka", [F, 12], F32, kind="ExternalInput")
    Wo_d = nc.dram_tensor("Wo", [P, D], F32, kind="ExternalInput")
    id_d = nc.dram_tensor("id12", [12, 12], F16, kind="ExternalInput")
    ones_d = nc.dram_tensor("ones12", [1, 12], F16, kind="ExternalInput")
    out_d = nc.dram_tensor("out", [S, F], F32, kind="ExternalOutput")

    with tile.TileContext(nc) as tc:
        with ExitStack() as ctx:
            cpool = ctx.enter_context(tc.tile_pool(name="const", bufs=1))
            ework = ctx.enter_context(tc.tile_pool(name="ework", bufs=1))
            ost = ctx.enter_context(tc.tile_pool(name="ost", bufs=3))
            psA = ctx.enter_context(tc.tile_pool(name="psA", bufs=2, space="PSUM"))
            psB = ctx.enter_context(tc.tile_pool(name="psB", bufs=2, space="PSUM"))
            psW = ctx.enter_context(tc.tile_pool(name="psW", bufs=2, space="PSUM"))
            pools = {"psA": psA, "psB": psB, "psW": psW, "ework": ework}

            # ---- constant/resident loads
            id_sb = cpool.tile([12, 12], F16, tag="id")
            nc.sync.dma_start(id_sb[:], id_d[:])
            ones_sb = cpool.tile([1, 12], F16, tag="ones")
            nc.sync.dma_start(ones_sb[:], ones_d[:])
            lm_sb = cpool.tile([1, S], F16, tag="lm")
            nc.sync.dma_start(lm_sb[:], lm_d[:])
            Aq_sb = cpool.tile([P, 12 * NF], F16, tag="Aq")
            for j in range(NF):
                nc.sync.dma_start(Aq_sb[:, 12 * j:12 * (j + 1)],
                                  Aq_d[P * j:P * (j + 1), :])

            xT_sb = []
            for j in range(NF):
                t = cpool.tile([P, S], F16, tag=f"xT{j}")
                nc.sync.dma_start(t[:], xT_d[P * j:P * (j + 1), :])
                xT_sb.append(t)
            x_sb = []
            for i in range(NS):
                t = cpool.tile([P, F], F16, tag=f"x{i}")
                nc.sync.dma_start(t[:], x_d[P * i:P * (i + 1), :])
                x_sb.append(t)

            def load_w(dram, name):
                tiles = []
                for j in range(NF):
                    t = cpool.tile([P, F], F16, tag=f"{name}{j}")
                    nc.sync.dma_start(t[:], dram[P * j:P * (j + 1), :])
                    tiles.append(t)
                return tiles

            Wkt_sb = load_w(Wkt_d, "Wkt")
            Wk_sb = load_w(Wk_d, "Wk")
            Wq_sb = load_w(Wq_d, "Wq")
            Wqt_sb = load_w(Wqt_d, "Wqt")
            Wka_sb = cpool.tile([P, 12 * NF], F32, tag="Wka")
            for j in range(NF):
                nc.sync.dma_start(Wka_sb[:, 12 * j:12 * (j + 1)],
                                  Wka_d[P * j:P * (j + 1), :])
            Wo_sb = cpool.tile([P, D], F32, tag="Wo")
            nc.sync.dma_start(Wo_sb[:], Wo_d[:])

            cst = {"xT_sb": xT_sb, "x_sb": x_sb, "lm_sb": lm_sb,
                   "ones_sb": ones_sb, "id_sb": id_sb}

            # ---- pass 1: query pooling
            if stage >= 2:
                _build_main(nc, tc, pools, cst, cpool, ework, ost, psA, psB, psW,
                            Aq_sb, Wq_sb, Wqt_sb, Wk_sb, Wkt_sb, Wka_sb, Wo_sb,
                            xT_sb, x_sb, id_sb, out_d, stage)
            else:
                M_sb = []
                for ft in range(NF):
                    m = ework.tile([P, F], F16, tag=f"M{ft}")
                    nc.vector.tensor_copy(m[:], Wq_sb[ft][:])
                    M_sb.append(m)
                for i in range(NS):
                    ops = psW.tile([P, F], F32, tag="wide")
                    for j in range(NF):
                        lhsT = xT_sb[j][:, P * i:P * (i + 1)]
                        nc.tensor.matmul(ops[:, 0:512], lhsT, M_sb[j][:, 0:512],
                                         start=(j == 0), stop=(j == NF - 1))
                        nc.tensor.matmul(ops[:, 512:F], lhsT, M_sb[j][:, 512:F],
                                         start=(j == 0), stop=(j == NF - 1))
                    o = ost.tile([P, F], F32, tag="outst")
                    nc.vector.tensor_copy(o[:], ops[:])
                    nc.sync.dma_start(out_d[P * i:P * (i + 1), :], o[:])

    nc.compile()
    return nc


def _build_main(nc, tc, pools, cst, cpool, ework, ost, psA, psB, psW,
                Aq_sb, Wq_sb, Wqt_sb, Wk_sb, Wkt_sb, Wka_sb, Wo_sb,
                xT_sb, x_sb, id_sb, out_d, stage):
    if True:
        if True:
            xw_q, inv_q = _emit_pool_pass(nc, tc, pools, cst,
                                          lambda j: Aq_sb[:, 12 * j:12 * (j + 1)], "q")
            qctx = _emit_ctx_extract(nc, tc, pools, cst, xw_q, inv_q, Wq_sb, "q")

            if stage == 2:
                M_sb = []
                for ft in range(NF):
                    m = ework.tile([P, F], F16, tag=f"M{ft}")
                    nc.vector.tensor_copy(m[:], Wq_sb[ft][:])
                    M_sb.append(m)
                _emit_pass3(nc, pools, ost, xT_sb, M_sb, out_d)
                return

            # ---- A_k = Wk @ (q_ctx * Wka)
            qWka = ework.tile([P, 12 * NF], F16, tag="qWka")
            nc.vector.tensor_tensor(
                qWka[:].rearrange("p (a b) -> p a b", a=NF),
                Wka_sb[:].rearrange("p (a b) -> p a b", a=NF),
                qctx[:, :, None].broadcast_to((P, NF, 12)),
                mybir.AluOpType.mult)
            Ak_ps = psA.tile([P, 12 * NF], F32, tag="sc")
            for ft in range(NF):
                for fc in range(NF):
                    nc.tensor.matmul(
                        Ak_ps[:, 12 * ft:12 * (ft + 1)],
                        Wkt_sb[fc][:, P * ft:P * (ft + 1)],
                        qWka[:, 12 * fc:12 * (fc + 1)],
                        start=(fc == 0), stop=(fc == NF - 1))
            Ak16 = ework.tile([P, 12 * NF], F16, tag="Ak16")
            nc.vector.tensor_copy(Ak16[:], Ak_ps[:])

            # ---- pass 2: key pooling (gated)
            xw_k, inv_k = _emit_pool_pass(nc, tc, pools, cst,
                                          lambda j: Ak16[:, 12 * j:12 * (j + 1)], "k")
            kc0 = _emit_ctx_extract(nc, tc, pools, cst, xw_k, inv_k, Wk_sb, "k")

            if stage == 3:
                M_sb = []
                for ft in range(NF):
                    m = ework.tile([P, F], F16, tag=f"M{ft}")
                    nc.vector.tensor_copy(m[:], Wq_sb[ft][:])
                    M_sb.append(m)
                _emit_pass3(nc, pools, ost, xT_sb, M_sb, out_d)
                return

            if stage == 5:
                kctx = ework.tile([P, NF], F32, tag="kctx_prod")
                nc.vector.tensor_tensor(kctx[:], qctx[:], kc0[:],
                                        mybir.AluOpType.mult)
                TWo = []
                for j in range(NF):
                    t = ework.tile([P, D], F16, tag=f"TWo{j}")
                    nc.vector.tensor_scalar_mul(t[:], Wo_sb[:], kctx[:, j:j + 1])
                    TWo.append(t)
                M_sb = []
                for ft in range(NF):
                    m = ework.tile([P, F], F16, tag=f"M{ft}")
                    nc.vector.tensor_copy(m[:], Wq_sb[ft][:])
                    nc.vector.tensor_copy(m[:, 0:D], TWo[ft][:])
                    M_sb.append(m)
                _emit_pass3(nc, pools, ost, xT_sb, M_sb, out_d)
                return

            if stage == 6:
                Wo16 = ework.tile([P, D], F16, tag="Wo16")
                nc.vector.tensor_copy(Wo16[:], Wo_sb[:])
                M_sb = []
                for ft in range(NF):
                    Mc = psW.tile([P, F], F32, tag="wide")
                    for h in range(H):
                        lo = 64 * (h % 2)
                        nc.tensor.matmul(
                            Mc[:, D * h:D * (h + 1)],
                            Wqt_sb[h // 2][lo:lo + D, P * ft:P * (ft + 1)],
                            Wo16[lo:lo + D, :], start=True, stop=True)
                    m = ework.tile([P, F], F16, tag=f"M{ft}")
                    nc.vector.tensor_add(m[:], Mc[:], Wq_sb[ft][:])
                    M_sb.append(m)
                _emit_pass3(nc, pools, ost, xT_sb, M_sb, out_d)
                return

            kctx = ework.tile([P, NF], F32, tag="kctx_prod")
            nc.vector.tensor_tensor(kctx[:], qctx[:], kc0[:], mybir.AluOpType.mult)

            # ---- M = Wq + concat_h(Wq[:, h] @ (k_ctx[h] . Wo))
            # R[j] is a block-diagonal [128,128] gated-Wo for the head pair
            # (2j, 2j+1): rows 0:64 scale Wo by kctx head 2j into cols 0:64,
            # rows 64:128 scale (stacked) Wo by head 2j+1 into cols 64:128.
            # Keeps every matmul operand at partition base 0 (the HW rejects
            # tile_position-offset matmuls that partition-offset slices emit).
            R_sb = []
            for j in range(NF):
                r = ework.tile([P, P], F16, tag=f"R{j}")
                nc.vector.memset(r[:], 0.0)
                nc.vector.tensor_scalar_mul(r[0:64, 0:64], Wo_sb[0:64, :],
                                            kctx[0:64, j:j + 1])
                nc.vector.tensor_scalar_mul(r[64:P, 64:P], Wo_sb[64:P, :],
                                            kctx[64:P, j:j + 1])
                R_sb.append(r)

            M_sb = []
            for ft in range(NF):
                Mc = psW.tile([P, F], F32, tag="wide")
                for j in range(NF):
                    nc.tensor.matmul(Mc[:, P * j:P * (j + 1)],
                                     Wqt_sb[j][:, P * ft:P * (ft + 1)],
                                     R_sb[j][:], start=True, stop=True)
                m = ework.tile([P, F], F16, tag=f"M{ft}")
                nc.vector.tensor_add(m[:], Mc[:], Wq_sb[ft][:])
                M_sb.append(m)

            # ---- pass 3: out = x @ M
            _emit_pass3(nc, pools, ost, xT_sb, M_sb, out_d)


def _emit_pass3(nc, pools, ost, xT_sb, M_sb, out_d):
    psW = pools["psW"]
    for i in range(NS):
        ops = psW.tile([P, F], F32, tag="wide")
        for j in range(NF):
            lhsT = xT_sb[j][:, P * i:P * (i + 1)]
            nc.tensor.matmul(ops[:, 0:512], lhsT, M_sb[j][:, 0:512],
                             start=(j == 0), stop=(j == NF - 1))
            nc.tensor.matmul(ops[:, 512:F], lhsT, M_sb[j][:, 512:F],
                             start=(j == 0), stop=(j == NF - 1))
        o = ost.tile([P, F], F32, tag="outst")
        nc.vector.tensor_copy(o[:], ops[:])
        nc.sync.dma_start(out_d[P * i:P * (i + 1), :], o[:])


def _get_program():
    if "nc" not in _prog_cache:
        _prog_cache["nc"] = build_program()
    return _prog_cache["nc"]


def _prep_core_inputs(xb, maskb, w16):
    lm = np.where(maskb > 0, np.float16(0), np.float16(-60000.0))[None, :]
    return {
        "xT": np.ascontiguousarray(xb.T).astype(np.float16),
        "x": xb.astype(np.float16),
        "lm": lm.astype(np.float16),
        **w16,
    }


def run(x, attn_mask, Wq, Wk, Wqa, Wka, Wo, trace=False):
    from concourse.bass_utils import run_bass_kernel_spmd

    nc = _get_program()
    w16 = {
        "Aq": (Wq @ Wqa).astype(np.float16),
        "Wq": Wq.astype(np.float16),
        "Wqt": np.ascontiguousarray(Wq.T).astype(np.float16),
        "Wk": Wk.astype(np.float16),
        "Wkt": np.ascontiguousarray(Wk.T).astype(np.float16),
        "Wka": Wka.astype(np.float32),
        "Wo": np.vstack([Wo, Wo]).astype(np.float32),
        "id12": np.eye(12, dtype=np.float16),
        "ones12": np.ones((1, 12), dtype=np.float16),
    }
    in_maps = [_prep_core_inputs(np.asarray(x[b]), np.asarray(attn_mask[b]), w16)
               for b in range(N_CORES)]
    res = run_bass_kernel_spmd(nc, in_maps, list(range(N_CORES)), trace=trace)
    out = np.stack([res.results[b]["out"] for b in range(N_CORES)])
    return out, res


def kernel(x, attn_mask, Wq, Wk, Wqa, Wka, Wo):
    out, _ = run(np.asarray(x, dtype=np.float32), np.asarray(attn_mask, dtype=np.float32),
                 np.asarray(Wq, dtype=np.float32), np.asarray(Wk, dtype=np.float32),
                 np.asarray(Wqa, dtype=np.float32), np.asarray(Wka, dtype=np.float32),
                 np.asarray(Wo, dtype=np.float32))
    return out
